# revision 32
# baseline (speedup 1.0000x reference)
"""Trainium2 Bass kernel: non-causal multi-head self-attention (B=1, T=4096,
D=1024, H=16) sharded over heads across 8 NeuronCores (2 heads per core).

Per-core dataflow (Tile-scheduled, hardware For_i loops to keep the
instruction stream small — this runtime pays heavily for long streams):
  prefix : DMA weights + all of x^T (bf16) into SBUF; ones columns for V_aug.
  loop 1 : over t-chunks: qT/kT = W_{q,k} @ x^T (heads stacked on partitions:
           h0 rows 0-63, h1 rows 64-127); V in natural [t, hd] layout (ones
           column appended) via x^T-stationary matmuls.
  loop 2 : over q-chunks: flash-style attention: S^T = K Q^T on PE (both
           heads row-packed via tile_position), one exp per k-block on
           ScalarE ([128,1024], scale=1/8 folded; no max-subtraction needed
           since |scores/8| < ~6), P^T @ V_aug accumulated in PSUM; ones
           column yields sumexp; normalize = DVE reciprocal (cross-quadrant
           partition 64 -> 0) + GpSimd partition_broadcast + DVE multiply.
           The output projection for the PREVIOUS q-chunk is folded into the
           same iteration (reads y^T slot i, writes outT cols shifted by one
           chunk) so its PE/DVE/DMA work hides under the exp-bound k-loop.
  epilog : projection of the last q-chunk.
Host: shards weights per core, sums the 8 partial projections, transposes.
"""

import os
import sys

for _p in ("/opt/trn_rl_repo", "/root/.axon_site/_ro/trn_rl_repo"):
    if os.path.isdir(_p) and _p not in sys.path:
        sys.path.insert(0, _p)

import numpy as np
import ml_dtypes

import concourse.bass as bass
import concourse.tile as tile
from concourse import bacc, mybir
from concourse.bass_utils import run_bass_kernel_spmd

BF16 = mybir.dt.bfloat16
F32 = mybir.dt.float32
NP_BF16 = ml_dtypes.bfloat16
ds = bass.ds
ts = bass.ts

T = 4096
D = 1024
HD = 64
N_CORES = 8
TC = 512            # t-chunk / q-chunk width
NTC = T // TC       # 8
NDB = D // 128      # 8 contraction blocks
NKB = T // 128      # 32 key blocks
VW = 2 * (HD + 1)   # 130: per-kblock V row = [v_h0(64), ones, v_h1(64), ones]
YSLOTS = NTC + 1    # y^T ring slots (slot 0 is a dummy for the sw pipeline)


def build_nc(reps: int = 1, phases: str = "12", p2mode: str = "e"):
    nc = bacc.Bacc("TRN2", target_bir_lowering=False, debug=False,
                   num_devices=N_CORES)
    xT = nc.dram_tensor("xT", [D, T], BF16, kind="ExternalInput").ap()
    wqkvT = nc.dram_tensor("wqkvT", [D, 384], BF16, kind="ExternalInput").ap()
    wpT = nc.dram_tensor("wpT", [HD, 2 * D], BF16, kind="ExternalInput").ap()
    # outS[p, slot, ec, w] = partial^T[ec*128 + p, (slot-1)*TC + w]; slot 0 dummy
    outS = nc.dram_tensor("outS", [128, YSLOTS, NDB, TC], F32,
                          kind="ExternalOutput").ap()

    # [p, tci, db, w] = xT[db*128 + p, tci*TC + w]
    x_r = xT.rearrange("(db p) (tc w) -> p tc db w", p=128, w=TC)
    wqkv_r = wqkvT.rearrange("(o p) e -> p o e", p=128)

    with tile.TileContext(nc) as tc:
        with (
            tc.tile_pool(name="consts", bufs=1) as consts,
            tc.tile_pool(name="pt_pool", bufs=3) as pt_pool,
            tc.tile_pool(name="norm_pool", bufs=3) as norm_pool,
            tc.tile_pool(name="out_pool", bufs=2) as out_pool,
            tc.tile_pool(name="mm_ps", bufs=2, space="PSUM") as mm_ps,
            tc.tile_pool(name="s_ps", bufs=2, space="PSUM") as s_ps,
            tc.tile_pool(name="o_ps", bufs=2, space="PSUM") as o_ps,
        ):
            wqkv_sb = consts.tile([128, NDB, 384], BF16)
            wp_sb = consts.tile([HD, 2 * D], BF16)
            xt_all = consts.tile([128, NTC * NDB * TC], BF16)
            qT_sb = consts.tile([128, T], BF16)
            kT_sb = consts.tile([128, T], BF16)
            vT_sb = consts.tile([128, T], BF16)
            v_tmp = consts.tile([128, NKB, 128], BF16)
            v_sb = consts.tile([128, NKB * VW], BF16)
            yt_sb = consts.tile([HD, 2 * YSLOTS * TC], BF16)
            ones64 = consts.tile([1, HD], F32)

            def body():
                nc.sync.dma_start(wqkv_sb[:], wqkv_r)
                nc.sync.dma_start(wp_sb[:], wpT)
                nc.sync.dma_start(
                    xt_all.rearrange("p (tc db w) -> p tc db w", db=NDB, w=TC),
                    x_r)
                v3 = v_sb.rearrange("p (k w) -> p k w", w=VW)
                nc.vector.memset(v3[:, :, HD:HD + 1], 1.0)
                nc.vector.memset(v3[:, :, 2 * HD + 1:VW], 1.0)
                nc.vector.memset(ones64[:], 1.0)

                # -------- loop 1: qkv projections --------
                if "1" not in phases:
                    return
                with tc.For_i(0, NTC, 1, staggered_reset=True) as i:
                    for ec, dst in ((0, qT_sb), (1, kT_sb), (2, vT_sb)):
                        ps_qk = mm_ps.tile([128, TC], F32, tag="mm", name="ps_qk")
                        for db in range(NDB):
                            nc.tensor.matmul(
                                ps_qk[:],
                                wqkv_sb[:, db, ts(ec, 128)],
                                xt_all[:, ds((i * NDB + db) * TC, TC)],
                                start=(db == 0), stop=(db == NDB - 1),
                            )
                        nc.vector.tensor_copy(dst[:, ds(i * TC, TC)], ps_qk[:])

                # V to natural [t, e] layout: XBAR transpose + gap-strided copy
                nc.sync.dma_start_transpose(v_tmp[:], vT_sb[:])
                nc.vector.tensor_copy(v3[:, :, 0:HD], v_tmp[:, :, 0:HD])
                nc.vector.tensor_copy(
                    v3[:, :, HD + 1:2 * HD + 1], v_tmp[:, :, HD:128])

                # -------- loop 2: attention (+ pipelined projection) --------
                if "2" not in phases:
                    return
                with tc.For_i(0, NTC, 1, staggered_reset=True) as i:
                    if p2mode not in ("a", "b"):
                        o0 = o_ps.tile([HD + 1, TC], F32, tag="o", name="o0")
                        o1 = o_ps.tile([HD + 1, TC], F32, tag="o", name="o1")
                    for kb in range(NKB):
                        s = s_ps.tile([128, 2 * TC], F32, name="s")
                        nc.tensor.matmul(
                            s[:, 0:TC],
                            kT_sb[0:HD, ts(kb, 128)],
                            qT_sb[0:HD, ds(i * TC, TC)],
                            start=True, stop=True,
                        )
                        nc.tensor.matmul(
                            s[:, TC:2 * TC],
                            kT_sb[HD:128, ts(kb, 128)],
                            qT_sb[HD:128, ds(i * TC, TC)],
                            tile_position=(64, 0),
                            start=True, stop=True,
                        )
                        if p2mode == "a":
                            continue
                        pt = pt_pool.tile([128, 2 * TC], BF16, name="pt")
                        nc.scalar.activation(
                            pt[:], s[:],
                            mybir.ActivationFunctionType.Exp, scale=0.125)
                        if p2mode == "b":
                            continue
                        for h, o in ((0, o0), (1, o1)):
                            nc.tensor.matmul(
                                o[:],
                                v_sb[:, kb * VW + h * (HD + 1):
                                     kb * VW + (h + 1) * (HD + 1)],
                                pt[:, ts(h, TC)],
                                start=(kb == 0), stop=(kb == NKB - 1),
                            )
                    for h, o in (() if p2mode in ("a", "b")
                                 else ((0, o0), (1, o1))):
                        osb = norm_pool.tile([HD + 1, TC], F32, tag="osb",
                                             name="osb")
                        nc.vector.tensor_copy(osb[:], o[:])
                        if p2mode == "c":
                            nc.vector.tensor_copy(
                                yt_sb[:, ds(h * YSLOTS * TC + (i + 1) * TC, TC)],
                                osb[0:HD, :])
                            continue
                        rec0 = norm_pool.tile([1, TC], F32, tag="rec",
                                              name="rec0")
                        nc.vector.reciprocal(rec0[:], osb[HD:HD + 1, :])
                        # broadcast 1/sumexp across partitions via K=1 matmul
                        bc = mm_ps.tile([128, TC], F32, tag="mm", name="bc")
                        nc.tensor.matmul(bc[0:HD, :], ones64[:], rec0[:],
                                         start=True, stop=True)
                        # write y^T into ring slot i+1
                        nc.vector.tensor_mul(
                            out=yt_sb[:, ds(h * YSLOTS * TC + (i + 1) * TC, TC)],
                            in0=osb[0:HD, :], in1=bc[0:HD, :])
                    # projection of the previous q-chunk (slot i; dummy at i=0)
                    if p2mode == "e":
                        ob_all = out_pool.tile([128, NDB, TC], F32,
                                               tag="ob", name="ob_all")
                        for ec in range(NDB):
                            pp = mm_ps.tile([128, TC], F32, tag="mm", name="pp")
                            for h in range(2):
                                nc.tensor.matmul(
                                    pp[:],
                                    wp_sb[:, h * D + ec * 128:
                                          h * D + (ec + 1) * 128],
                                    yt_sb[:, ds(h * YSLOTS * TC + i * TC, TC)],
                                    start=(h == 0), stop=(h == 1),
                                )
                            nc.vector.tensor_copy(ob_all[:, ec, :], pp[:])
                        nc.sync.dma_start(outS[:, ds(i, 1), :, :],
                                          ob_all[:, None, :, :])

                # -------- epilogue: projection of the last q-chunk --------
                ob_all_e = out_pool.tile([128, NDB, TC], F32, tag="ob",
                                         name="ob_all_e")
                for ec in range(NDB):
                    pp = mm_ps.tile([128, TC], F32, tag="mm", name="pp_e")
                    for h in range(2):
                        nc.tensor.matmul(
                            pp[:],
                            wp_sb[:, h * D + ec * 128:h * D + (ec + 1) * 128],
                            yt_sb[:, (h * YSLOTS + NTC) * TC:
                                  (h * YSLOTS + NTC + 1) * TC],
                            start=(h == 0), stop=(h == 1),
                        )
                    nc.vector.tensor_copy(ob_all_e[:, ec, :], pp[:])
                nc.sync.dma_start(outS[:, NTC:NTC + 1, :, :],
                                  ob_all_e[:, None, :, :])

            if reps == 1:
                body()
            else:
                with tc.For_i(0, reps, 1):
                    body()

    nc.compile()
    return nc


_NC_CACHE = {}


def _get_nc(reps: int = 1):
    if reps not in _NC_CACHE:
        _NC_CACHE[reps] = build_nc(reps)
    return _NC_CACHE[reps]


def make_in_maps(x, w_attn, w_proj):
    x = np.asarray(x, dtype=np.float32)
    w_attn = np.asarray(w_attn, dtype=np.float32)
    w_proj = np.asarray(w_proj, dtype=np.float32)
    xT_bf = np.ascontiguousarray(x[0].T).astype(NP_BF16)
    in_maps = []
    for c in range(N_CORES):
        r0 = 2 * c * HD          # first row of this core's head pair
        wq = w_attn[r0:r0 + 128]
        wk = w_attn[D + r0:D + r0 + 128]
        wv = w_attn[2 * D + r0:2 * D + r0 + 128]
        wqkvT = np.ascontiguousarray(
            np.concatenate([wq, wk, wv], 0).T).astype(NP_BF16)
        wp = w_proj[:, r0:r0 + 128]
        wpT = np.ascontiguousarray(
            np.concatenate([wp[:, 0:HD].T, wp[:, HD:128].T], 1)).astype(NP_BF16)
        in_maps.append({"xT": xT_bf, "wqkvT": wqkvT, "wpT": wpT})
    return in_maps


def kernel(x, w_attn, w_proj):
    in_maps = make_in_maps(x, w_attn, w_proj)
    nc = _get_nc(1)
    res = run_bass_kernel_spmd(nc, in_maps, core_ids=list(range(N_CORES)))
    total = np.zeros((D, T), np.float64)
    for c in range(N_CORES):
        o = res.results[c]["outS"]  # [128, YSLOTS, NDB, TC]
        o = o.transpose(2, 0, 1, 3).reshape(D, YSLOTS * TC)[:, TC:]
        total += o
    return np.ascontiguousarray(total.T.astype(np.float32)).reshape(1, T, D)
